# revision 12
# baseline (speedup 1.0000x reference)
"""Bipartite GNN message-passing layer on 8 Trainium2 NeuronCores.

Strategy (per spec sharding hint): shard target nodes across the 8 cores
(6250 targets/core/direction); partition edges by target so the
scatter-mean is local to each core; replicate source features + weights.

Per core, per direction:
  - edges sorted by target, grouped into 128-target tiles;
  - per-edge source rows gathered HBM->SBUF in bf16 via the Q7 dma_gather
    custom op (int16 indices => edges split by source half);
  - segment-sum runs on the TensorEngine: a one-hot scatter matrix S
    (built on DVE with is_equal vs an iota row) times the gathered
    message block, accumulated into a PSUM tile per 128 targets;
  - mean+residual fused in one DVE op reading PSUM, PE transpose,
    fp32r dense matmul (+ bias via a K=1 matmul), relu+layernorm on ACT.
"""

import os
import sys

if "/opt/trn_rl_repo" not in sys.path:
    sys.path.insert(0, "/opt/trn_rl_repo")

from contextlib import ExitStack

import ml_dtypes
import numpy as np

# test-only hooks (harness leaves these off)
_TRACE = bool(os.environ.get("BGK_TRACE"))
_DIR_LIMIT = int(os.environ.get("BGK_DIR_LIMIT", "2"))
_SUPER_LIMIT = int(os.environ.get("BGK_SUPER_LIMIT", "99"))
last_result = None

D = 256
NNODE = 50000
N_CORES = 8
TPC = NNODE // N_CORES  # targets per core
TILE = 128
NT = (TPC + TILE - 1) // TILE  # 49 target tiles per core
SPLIT = 32768  # int16 index reach; edges split by source half
SUPER = 7  # tiles per gather supertile (7 x 7 = 49)
GATHER_BF16 = True

_prog_cache = {}


def _wrap_idx(idx):
    """dma_gather index layout: edge i -> [i % 16, i // 16], replicated
    across the 8 Q7-core partition groups."""
    assert len(idx) % 16 == 0
    w = idx.reshape(-1, 16).T.astype(np.int16)  # [16, n/16]
    return np.tile(w, (8, 1))  # [128, n/16]


def _prep_direction(src, tgt):
    """Host-side shard/sort/pad. Returns (nblk[NT][2] uniform block counts,
    per-core dict of arrays)."""
    deg = np.bincount(tgt, minlength=NNODE).astype(np.float64)
    recip_full = (1.0 / np.maximum(deg, 1.0)).astype(np.float32)

    order = np.argsort(tgt, kind="stable")
    s_all, t_all = src[order], tgt[order]
    cb = np.searchsorted(t_all, np.arange(N_CORES + 1) * TPC)

    segs = []  # [core][tile] -> ((srcA, tgtA), (srcB, tgtB))
    for c in range(N_CORES):
        s = s_all[cb[c] : cb[c + 1]]
        t = t_all[cb[c] : cb[c + 1]] - c * TPC
        tb = np.searchsorted(t, np.arange(NT + 1) * TILE)
        tiles = []
        for ti in range(NT):
            ss = s[tb[ti] : tb[ti + 1]]
            tt = t[tb[ti] : tb[ti + 1]] - ti * TILE
            mA = ss < SPLIT
            tiles.append(((ss[mA], tt[mA]), (ss[~mA] - SPLIT, tt[~mA])))
        segs.append(tiles)

    nblk = np.zeros((NT, 2), np.int64)
    for ti in range(NT):
        for g in range(2):
            mx = max(len(segs[c][ti][g][0]) for c in range(N_CORES))
            nblk[ti, g] = -(-mx // 128)
        if nblk[ti].sum() == 0:
            nblk[ti, 0] = 1

    cores = []
    for c in range(N_CORES):
        idx_cat = [[], []]
        ltgt_cat = [[], []]
        for ti in range(NT):
            for g in range(2):
                n = nblk[ti, g] * 128
                ss, tt = segs[c][ti][g]
                idx = np.zeros(n, np.int64)
                lt = np.full(n, -1.0, np.float32)
                idx[: len(ss)] = ss
                lt[: len(tt)] = tt.astype(np.float32)
                idx_cat[g].append(idx)
                ltgt_cat[g].append(lt)
        idx_g, ltgt_g = [], []
        gdt_np = ml_dtypes.bfloat16 if GATHER_BF16 else np.float32
        for g in range(2):
            idx = np.concatenate(idx_cat[g])
            lt = np.concatenate(ltgt_cat[g])
            idx_g.append(_wrap_idx(idx))
            # [128, totblk]
            ltgt_g.append(lt.reshape(-1, 128).T.astype(gdt_np))
        recip = np.zeros((128, NT), np.float32)
        flat = recip_full[c * TPC : (c + 1) * TPC]
        pad = np.zeros(NT * 128, np.float32)
        pad[: len(flat)] = flat
        recip[:] = pad.reshape(NT, 128).T
        cores.append(
            {
                "idxA": idx_g[0],
                "idxB": idx_g[1],
                "ltgtA": ltgt_g[0],
                "ltgtB": ltgt_g[1],
                "recip": recip,
            }
        )
    return nblk, cores


def _build_program(nblk_u, nblk_i, apply_gamma_beta):
    import concourse.bass as bass
    import concourse.tile as tile
    from concourse import bacc, mybir

    f32 = mybir.dt.float32
    f32r = mybir.dt.float32r
    bf16 = mybir.dt.bfloat16
    i16 = mybir.dt.int16
    gdt = bf16 if GATHER_BF16 else f32r
    Alu = mybir.AluOpType
    Act = mybir.ActivationFunctionType

    nc = bacc.Bacc("TRN2", target_bir_lowering=False, debug=False,
                   num_devices=N_CORES)

    def din(name, shape, dt):
        return nc.dram_tensor(name, shape, dt, kind="ExternalInput").ap()

    def dout(name, shape, dt):
        return nc.dram_tensor(name, shape, dt, kind="ExternalOutput").ap()

    dirs = []
    for d, nblk in (("u", nblk_u), ("i", nblk_i)):
        totblk = [int(nblk[:, g].sum()) for g in range(2)]
        dirs.append(
            {
                "name": d,
                "nblk": nblk,
                "totblk": totblk,
                "src16": din(f"src16_{d}", [NNODE, D], gdt),
                "x": din(f"x_{d}", [TPC, D], f32),
                "W": din(f"W_{d}", [2, 128, D], f32r),
                "bias": din(f"bias_{d}", [1, D], f32r),
                "idx": [
                    din(f"idx{g}_{d}", [128, totblk[g] * 8], i16) for g in range(2)
                ],
                "ltgt": [
                    din(f"ltgt{g}_{d}", [128, totblk[g]], gdt) for g in range(2)
                ],
                "recip": din(f"recip_{d}", [128, NT], f32),
                "out": dout(f"out_{d}", [TPC, D], f32),
            }
        )
    iota_d = din("iota", [128, 128], gdt)
    ident_d = din("ident", [128, 128], f32)
    ones_d = din("ones", [1, 128], f32r)
    if apply_gamma_beta:
        gamma_d = din("gamma_rep", [128, D], f32)
        beta_d = din("beta_rep", [128, D], f32)

    with tile.TileContext(nc) as tc, ExitStack() as ctx:
        consts = ctx.enter_context(tc.tile_pool(name="consts", bufs=1))
        meta = ctx.enter_context(tc.tile_pool(name="meta", bufs=1))
        msgs_p = ctx.enter_context(tc.tile_pool(name="msgs", bufs=2))
        s_p = ctx.enter_context(tc.tile_pool(name="s", bufs=3))
        work = ctx.enter_context(tc.tile_pool(name="work", bufs=3))
        stat = ctx.enter_context(tc.tile_pool(name="stat", bufs=4))
        psum_a = ctx.enter_context(tc.tile_pool(name="psa", bufs=2, space="PSUM"))
        psum_t = ctx.enter_context(tc.tile_pool(name="pst", bufs=2, space="PSUM"))
        psum_y = ctx.enter_context(tc.tile_pool(name="psy", bufs=2, space="PSUM"))

        iota_t = consts.tile([128, 128], gdt)
        nc.sync.dma_start(iota_t[:], iota_d[:])
        ident_t = consts.tile([128, 128], f32)
        nc.sync.dma_start(ident_t[:], ident_d[:])
        ones_t = consts.tile([1, 128], f32r)
        nc.sync.dma_start(ones_t[:], ones_d[:])
        if apply_gamma_beta:
            gamma_t = consts.tile([128, D], f32)
            nc.sync.dma_start(gamma_t[:], gamma_d[:])
            beta_t = consts.tile([128, D], f32)
            nc.sync.dma_start(beta_t[:], beta_d[:])

        for dd in dirs[:_DIR_LIMIT]:
            d = dd["name"]
            nblk = dd["nblk"]
            W_t = meta.tile([128, 2, D], f32r, name=f"W_{d}")
            for h in range(2):
                nc.sync.dma_start(W_t[:, h, :], dd["W"][h])
            bias_t = meta.tile([1, D], f32r, name=f"bias_{d}")
            nc.sync.dma_start(bias_t[:], dd["bias"][:])
            recip_t = meta.tile([128, NT], f32, name=f"recip_{d}")
            nc.sync.dma_start(recip_t[:], dd["recip"][:])
            idx_t, ltgt_t = [], []
            for g in range(2):
                it = meta.tile([128, dd["totblk"][g] * 8], i16, name=f"idx{g}_{d}")
                nc.sync.dma_start(it[:], dd["idx"][g][:])
                idx_t.append(it)
                lt = meta.tile([128, dd["totblk"][g]], gdt, name=f"ltgt{g}_{d}")
                nc.sync.dma_start(lt[:], dd["ltgt"][g][:])
                ltgt_t.append(lt)

            # supertile boundaries in tiles
            sup_bounds = list(range(0, NT, SUPER)) + [NT]
            blk_off = np.concatenate(
                [np.zeros((2, 1), np.int64), np.cumsum(nblk.T, axis=1)], axis=1
            )  # [2, NT+1] block offsets per group

            for si in range(min(len(sup_bounds) - 1, _SUPER_LIMIT)):
                t0, t1 = sup_bounds[si], sup_bounds[si + 1]
                msgs = []
                for g in range(2):
                    b0, b1 = int(blk_off[g, t0]), int(blk_off[g, t1])
                    nb = b1 - b0
                    if nb == 0:
                        msgs.append((None, b0))
                        continue
                    m = msgs_p.tile([128, nb, D], gdt, tag=f"msgs{g}",
                                    name=f"msgs{g}_{d}_{si}")
                    src_view = (
                        dd["src16"][0:SPLIT] if g == 0 else dd["src16"][SPLIT:NNODE]
                    )
                    nc.gpsimd.dma_gather(
                        m[:],
                        src_view,
                        idx_t[g][:, b0 * 8 : b1 * 8],
                        num_idxs=nb * 128,
                        num_idxs_reg=nb * 128,
                        elem_size=D,
                        # single-packet mode caps at 64 descs/engine (1024
                        # idxs); bigger calls hang the device
                        single_packet=(nb * 128 <= 1024),
                    )
                    msgs.append((m, b0))

                for ti in range(t0, t1):
                    nt = min(TILE, TPC - ti * TILE)
                    agg = psum_a.tile([128, D], f32, tag="agg", name=f"agg_{d}_{ti}")
                    # scatter matmuls over this tile's blocks (groups A then B)
                    tot_tile_blocks = int(nblk[ti, 0] + nblk[ti, 1])
                    done = 0
                    for g in range(2):
                        nb = int(nblk[ti, g])
                        if nb == 0:
                            continue
                        m, b0 = msgs[g]
                        lo = int(blk_off[g, ti])
                        S = s_p.tile([128, nb, 128], gdt, tag="S",
                                     name=f"S{g}_{d}_{ti}")
                        nc.vector.tensor_tensor(
                            S[:],
                            iota_t[:].unsqueeze(1).broadcast_to([128, nb, 128]),
                            ltgt_t[g][:, lo : lo + nb]
                            .unsqueeze(2)
                            .broadcast_to([128, nb, 128]),
                            Alu.is_equal,
                        )
                        for k in range(nb):
                            nc.tensor.matmul(
                                agg[:],
                                lhsT=S[:, k, :],
                                rhs=m[:, lo - b0 + k, :],
                                start=(done == 0),
                                stop=(done == tot_tile_blocks - 1),
                            )
                            done += 1

                    # xm = agg * recip + x   (DVE reads PSUM)
                    x_t = work.tile([128, D], f32, tag="x", name=f"x_{d}_{ti}")
                    nc.sync.dma_start(
                        x_t[:nt, :], dd["x"][ti * TILE : ti * TILE + nt]
                    )
                    xm = work.tile([128, D], f32, tag="xm", name=f"xm_{d}_{ti}")
                    nc.vector.scalar_tensor_tensor(
                        xm[:],
                        agg[:],
                        recip_t[:, ti : ti + 1],
                        x_t[:],
                        Alu.mult,
                        Alu.add,
                    )
                    # transpose xm -> [d, t] halves
                    tr = psum_t.tile([128, 2, 128], f32, tag="tr",
                                     name=f"tr_{d}_{ti}")
                    for h in range(2):
                        nc.tensor.transpose(
                            tr[:, h, :], xm[:, h * 128 : (h + 1) * 128], ident_t[:]
                        )
                    xmT = work.tile([128, 2, 128], f32r, tag="xmT",
                                    name=f"xmT_{d}_{ti}")
                    nc.vector.tensor_copy(xmT[:], tr[:])
                    # dense: y = bias + xm @ W
                    y_ps = psum_y.tile([128, D], f32, tag="y", name=f"y_{d}_{ti}")
                    nc.tensor.matmul(
                        y_ps[:], lhsT=ones_t[:], rhs=bias_t[:],
                        start=True, stop=False,
                    )
                    for h in range(2):
                        nc.tensor.matmul(
                            y_ps[:], lhsT=xmT[:, h, :], rhs=W_t[:, h, :],
                            start=False, stop=(h == 1),
                        )
                    # relu + LN
                    y_relu = work.tile([128, D], f32, tag="yr", name=f"yr_{d}_{ti}")
                    s1 = stat.tile([128, 1], f32, tag="s1", name=f"s1_{d}_{ti}")
                    nc.scalar.activation(y_relu[:], y_ps[:], Act.Relu,
                                         accum_out=s1[:])
                    sq = work.tile([128, D], f32, tag="sq", name=f"sq_{d}_{ti}")
                    s2 = stat.tile([128, 1], f32, tag="s2", name=f"s2_{d}_{ti}")
                    nc.scalar.activation(sq[:], y_relu[:], Act.Square,
                                         accum_out=s2[:])
                    mu = stat.tile([128, 1], f32, tag="mu", name=f"mu_{d}_{ti}")
                    nc.vector.tensor_scalar(mu[:], s1[:], 1.0 / D, None, Alu.mult)
                    msq = stat.tile([128, 1], f32, tag="msq", name=f"msq_{d}_{ti}")
                    # msq = mu*mu - eps
                    nc.vector.tensor_scalar(
                        msq[:], mu[:], mu[:], 1e-5, Alu.mult, Alu.subtract
                    )
                    var = stat.tile([128, 1], f32, tag="var", name=f"var_{d}_{ti}")
                    # var = s2/D - (mu*mu - eps) = var_true + eps
                    nc.vector.tensor_scalar(
                        var[:], s2[:], 1.0 / D, msq[:], Alu.mult, Alu.subtract
                    )
                    rv = stat.tile([128, 1], f32, tag="rv", name=f"rv_{d}_{ti}")
                    nc.vector.reciprocal(rv[:], var[:])
                    rstd = stat.tile([128, 1], f32, tag="rstd", name=f"rstd_{d}_{ti}")
                    nc.scalar.activation(rstd[:], rv[:], Act.Sqrt)
                    shift = stat.tile([128, 1], f32, tag="shift",
                                      name=f"shift_{d}_{ti}")
                    # shift = -mu * rstd
                    nc.vector.tensor_scalar(
                        shift[:], mu[:], rstd[:], -1.0, Alu.mult, Alu.mult
                    )
                    out_t = work.tile([128, D], f32, tag="out", name=f"o_{d}_{ti}")
                    nc.scalar.activation(
                        out_t[:], y_relu[:], Act.Identity,
                        bias=shift[:], scale=rstd[:],
                    )
                    if apply_gamma_beta:
                        nc.vector.tensor_tensor(
                            out_t[:], out_t[:], gamma_t[:], Alu.mult
                        )
                        nc.vector.tensor_tensor(
                            out_t[:], out_t[:], beta_t[:], Alu.add
                        )
                    nc.sync.dma_start(
                        dd["out"][ti * TILE : ti * TILE + nt], out_t[:nt, :]
                    )

    nc.compile()
    return nc


def kernel(
    user_features,
    item_features,
    user_item_edge_index,
    item_user_edge_index,
    Wu,
    bu,
    Wi,
    bi,
    gamma,
    beta,
):
    from concourse.bass_utils import run_bass_kernel_spmd

    uf = np.asarray(user_features, np.float32)
    itf = np.asarray(item_features, np.float32)
    ui = np.asarray(user_item_edge_index)
    iu = np.asarray(item_user_edge_index)
    Wu = np.asarray(Wu, np.float32)
    Wi = np.asarray(Wi, np.float32)
    bu = np.asarray(bu, np.float32)
    bi = np.asarray(bi, np.float32)
    gamma_np = np.asarray(gamma, np.float32)
    beta_np = np.asarray(beta, np.float32)

    gdt_np = ml_dtypes.bfloat16 if GATHER_BF16 else np.float32

    # direction "u": targets are users, sources are items
    nblk_u, cores_u = _prep_direction(
        iu[0].astype(np.int64), iu[1].astype(np.int64)
    )
    # direction "i": targets are items, sources are users
    nblk_i, cores_i = _prep_direction(
        ui[0].astype(np.int64), ui[1].astype(np.int64)
    )

    apply_gb = not (np.all(gamma_np == 1.0) and np.all(beta_np == 0.0))

    key = (nblk_u.tobytes(), nblk_i.tobytes(), apply_gb, _DIR_LIMIT, _SUPER_LIMIT)
    if key not in _prog_cache:
        _prog_cache[key] = _build_program(nblk_u, nblk_i, apply_gb)
    nc = _prog_cache[key]

    iota = np.tile(np.arange(128, dtype=np.float32)[None, :], (128, 1))
    ident = np.eye(128, dtype=np.float32)
    ones = np.ones((1, 128), np.float32)
    src16_u = itf.astype(gdt_np)  # sources for direction u are items
    src16_i = uf.astype(gdt_np)

    in_maps = []
    for c in range(N_CORES):
        im = {
            "src16_u": src16_u,
            "src16_i": src16_i,
            "x_u": np.ascontiguousarray(uf[c * TPC : (c + 1) * TPC]),
            "x_i": np.ascontiguousarray(itf[c * TPC : (c + 1) * TPC]),
            "W_u": Wu.reshape(2, 128, D),
            "W_i": Wi.reshape(2, 128, D),
            "bias_u": bu.reshape(1, D),
            "bias_i": bi.reshape(1, D),
            "recip_u": cores_u[c]["recip"],
            "recip_i": cores_i[c]["recip"],
            "idx0_u": cores_u[c]["idxA"],
            "idx1_u": cores_u[c]["idxB"],
            "idx0_i": cores_i[c]["idxA"],
            "idx1_i": cores_i[c]["idxB"],
            "ltgt0_u": cores_u[c]["ltgtA"],
            "ltgt1_u": cores_u[c]["ltgtB"],
            "ltgt0_i": cores_i[c]["ltgtA"],
            "ltgt1_i": cores_i[c]["ltgtB"],
            "iota": iota.astype(gdt_np),
            "ident": ident,
            "ones": ones,
        }
        if apply_gb:
            im["gamma_rep"] = np.tile(gamma_np[None, :], (128, 1))
            im["beta_rep"] = np.tile(beta_np[None, :], (128, 1))
        in_maps.append(im)

    res = run_bass_kernel_spmd(nc, in_maps, list(range(N_CORES)), trace=_TRACE)
    global last_result
    last_result = res
    u_new = np.concatenate([res.results[c]["out_u"] for c in range(N_CORES)])
    i_new = np.concatenate([res.results[c]["out_i"] for c in range(N_CORES)])
    return (u_new, i_new)
